# revision 94
# baseline (speedup 1.0000x reference)
"""Trainium2 kernel for nn_Contrast: contrastive loss over a 10000x10000
exp-cosine-similarity matrix, sharded by rows across 8 NeuronCores.

Device pipeline per core (1280-row slice, 10 strips of 128 rows):
  PE:  fp8e4m3 DoubleRow matmuls (K=8 split as 2 k-tiles of 4) compute
       logits*32 into PSUM at 0.5 model-cycles/col. One-hot matmuls reduce
       each strip's exp tiles over rows into a persistent colsum PSUM bank:
       paired DoubleRow one-hots for the fp8 span, plain one-hots for the
       bf16 span.
  ACT: exp(psum/32) -> fp8 et for 11 of 20 tiles per strip; accum_out gives
       f32 row-sum partials for free.
  DVE: Schraudolph exp for the other 9 tiles: tensor_scalar computes
       round(psum*A + B) as int16 whose bits ARE the bf16 exp approximation
       (rel err ~3%, averages out across the 10k-col sums); a second
       tensor_scalar (et*1+0 -> scratch) with accum_out row-sums those bf16
       values at the 4-elem/cycle all-SBUF rate.

PSUM layout (8 banks of 512 f32 cols): banks 0-3 = ACT window, banks 4-6 =
DVE window, bank 7 = persistent colsum accumulator. Tiles are interleaved
A4 D3 A4 D3 A3 D3 within each strip so both consumers chase the ring
concurrently; consecutive same-engine blocks reuse the same window with a
one-block pipeline lag.

Host: 8->8->8 MLP projection of both views (tiny), fp8 operand prep, exact
diag dots, gather + pad correction + log/mean finalize.
"""

import numpy as np

import concourse.bass as bass
import concourse.bacc as bacc
import concourse.mybir as mybir
import concourse.tile as tile
from concourse.bass_utils import run_bass_kernel_spmd

TAU = 0.5
LAM = 0.5
EPS = 1e-8

N = 10000
D = 8
NCORES = 8
RPC = 1280                 # rows per core (8*1280 = 10240, 240 zero pad rows)
NSTRIP = RPC // 128        # 10
ROW_PAD = NCORES * RPC - N  # 240 zero lhs rows in core 7

SA, SB = 8.0, 4.0          # fp8 operand scales; psum = 32 * logit
PSCALE = SA * SB
AS16 = float(128.0 / (PSCALE * np.log(2.0)))  # bf16 schraudolph slope
B16 = 16250.0              # 127*128 - 6: bf16 schraudolph offset (calibrated)

NT = 20                    # 512-col tiles per strip (last = 272)
TILE_W = [512] * 19 + [272]

# tile ownership: ACT window = psum banks 0-3, DVE window = banks 4-6
A_TILES = [0, 1, 2, 3, 7, 8, 9, 10, 14, 15, 16]   # 11 tiles, 5632 cols
D_TILES = [4, 5, 6, 11, 12, 13, 17, 18, 19]       # 9 tiles, 4368 cols

# host rhs layout: A tiles first so strip-0's ACT gate needs only the first
# rhs DMA chunk
RHS_ORDER = A_TILES + D_TILES
TILE_OFF = [0] * NT        # per-tile offset in the reordered rhs
_o = 0
for _t in RHS_ORDER:
    TILE_OFF[_t] = _o
    _o += 2 * TILE_W[_t]
RHS_LEN = _o               # 20000
RHS_SPLIT = 2 * 512 * len(A_TILES)  # 11264: end of the A-tile chunk
A_W = 512 * len(A_TILES)    # 5632
D_W = 8 * 512 + 272         # 4368
CB = 3584                   # colsum bank (psum cols 3584:4096)

# ACT runs (a-index ranges) and DVE runs (d-index ranges), per strip
A_RUNS = [(0, 4), (4, 4), (8, 3)]
D_RUNS = [(0, 3), (3, 3), (6, 3)]

f32 = mybir.dt.float32
bf16 = mybir.dt.bfloat16
fp8 = mybir.dt.float8e4
i16 = mybir.dt.int16


def _build_nc():
    nc = bacc.Bacc(None)
    lhsT = nc.dram_tensor("lhsT", [4, 2 * 128 * NSTRIP], fp8, kind="ExternalInput")
    rhsT = nc.dram_tensor("rhsT", [4, RHS_LEN], fp8, kind="ExternalInput")
    ebdr = nc.dram_tensor("ebdr", [128, 64 * 5], fp8, kind="ExternalInput")
    ebs8 = nc.dram_tensor("ebs8", [128, 20], fp8, kind="ExternalInput")
    ebbf = nc.dram_tensor("ebbf", [128, 20 * 9], bf16, kind="ExternalInput")
    out_rows = nc.dram_tensor("out_rows", [128, 4 * NSTRIP], f32, kind="ExternalOutput")
    out_colsum = nc.dram_tensor("out_colsum", [20, 512], f32, kind="ExternalOutput")

    with tile.TileContext(nc) as tc:
        with (
            tc.tile_pool(name="inp", bufs=1) as inp_pool,
            tc.tile_pool(name="eta", bufs=3) as etA_pool,
            tc.tile_pool(name="etd", bufs=3) as etD_pool,
            tc.tile_pool(name="persist", bufs=1) as persist_pool,
            tc.tile_pool(name="pring", bufs=1, space="PSUM") as pring_pool,
        ):
            lhsT_sb = inp_pool.tile([4, 2 * 128 * NSTRIP], fp8)
            rhsT_sb = inp_pool.tile([4, RHS_LEN], fp8)
            ebdr_sb = inp_pool.tile([128, 64 * 5], fp8)
            ebs8_sb = inp_pool.tile([128, 20], fp8)
            ebbf_sb = inp_pool.tile([128, 20 * 9], bf16)

            nc.sync.dma_start(out=lhsT_sb[:], in_=lhsT[:])
            # rhs in two chunks: A tiles land first (strip-0 gate), D follow
            nc.sync.dma_start(out=rhsT_sb[:, :RHS_SPLIT], in_=rhsT[:, :RHS_SPLIT])
            nc.sync.dma_start(out=rhsT_sb[:, RHS_SPLIT:], in_=rhsT[:, RHS_SPLIT:])
            nc.sync.dma_start(out=ebdr_sb[:], in_=ebdr[:])
            nc.sync.dma_start(out=ebs8_sb[:], in_=ebs8[:])
            nc.sync.dma_start(out=ebbf_sb[:], in_=ebbf[:])

            rows_all = persist_pool.tile([128, 4 * NSTRIP], f32)
            scratch = persist_pool.tile([128, D_W], bf16)
            colsum_sb = persist_pool.tile([20, 512], f32)
            ps = pring_pool.tile([128, 4096], f32)

            first_oh = [True]
            pending_oh = []

            def emit_onehots_a(etA, pairs=range(5), single=True):
                for p in pairs:  # fp8 DoubleRow pairs over A-tiles
                    nc.tensor.matmul(
                        ps[0:32, CB : CB + 512],
                        ebdr_sb[:, 64 * p : 64 * p + 64].rearrange(
                            "k (t m) -> k t m", t=2
                        ),
                        etA[:, 1024 * p : 1024 * p + 1024].rearrange(
                            "q (t n) -> q t n", t=2
                        ),
                        start=first_oh[0], stop=False,
                        perf_mode=mybir.MatmulPerfMode.DoubleRow,
                        skip_group_check=True,
                    )
                    first_oh[0] = False
                if single:
                    # last A-tile: plain fp8 one-hot
                    nc.tensor.matmul(
                        ps[0:20, CB : CB + 512],
                        ebs8_sb[:, 0:20],
                        etA[:, 5120:5632],
                        start=False, stop=False,
                        skip_group_check=True,
                    )

            def emit_onehots_d(etD):
                # D-tiles: plain bf16 one-hots over etD
                for j in range(9):
                    w = TILE_W[D_TILES[j]]
                    nc.tensor.matmul(
                        ps[0:20, CB : CB + w],
                        ebbf_sb[:, 20 * j : 20 * j + 20],
                        etD[:, 512 * j : 512 * j + w],
                        start=False, stop=False,
                        skip_group_check=True,
                    )

            def emit_onehots(etA, etD):
                emit_onehots_a(etA)
                emit_onehots_d(etD)

            for r in range(NSTRIP):
                lh = lhsT_sb[:, 256 * r : 256 * r + 256].rearrange(
                    "k (t m) -> k t m", t=2
                )
                etA = etA_pool.tile([128, A_W], fp8, name=f"etA{r % 3}", tag="etA")
                etD = etD_pool.tile([128, D_W], bf16, name=f"etD{r % 3}", tag="etD")

                def main(t, pcol, w):
                    nc.tensor.matmul(
                        ps[:, pcol : pcol + w],
                        lh,
                        rhsT_sb[:, TILE_OFF[t] : TILE_OFF[t] + 2 * w].rearrange(
                            "k (t2 n) -> k t2 n", t2=2
                        ),
                        start=True, stop=True,
                        perf_mode=mybir.MatmulPerfMode.DoubleRow,
                        skip_group_check=True,
                    )

                def mains_a(ais):
                    for ai in ais:
                        main(A_TILES[ai], 512 * (ai % 4), 512)

                def mains_d(dis):
                    for di in dis:
                        main(D_TILES[di], 2048 + 512 * (di % 3), TILE_W[D_TILES[di]])

                # emission order ~ expected readiness: A-block k+1 unblocks at
                # ACT-run k end; D-block k+1 at DVE-run k end (earlier). t3's
                # bank 3 is freed one ACT-run earlier than banks 0-2, so it
                # leads the A1 block.
                def act_run(j):
                    a0, nt_ = A_RUNS[j]
                    w = 512 * nt_
                    nc.scalar.activation(
                        etA[:, 512 * a0 : 512 * a0 + w],
                        ps[:, 512 * (a0 % 4) : 512 * (a0 % 4) + w],
                        mybir.ActivationFunctionType.Exp,
                        scale=1.0 / PSCALE,
                        accum_out=rows_all[:, 3 * r + j : 3 * r + j + 1],
                    )

                def dve_run(j):
                    d0, nt_ = D_RUNS[j]
                    w = sum(TILE_W[D_TILES[d0 + k]] for k in range(nt_))
                    nc.vector.tensor_scalar(
                        out=etD[:, 512 * d0 : 512 * d0 + w].bitcast(i16),
                        in0=ps[:, 2048 + 512 * (d0 % 3) : 2048 + 512 * (d0 % 3) + w],
                        scalar1=AS16, scalar2=B16,
                        op0=mybir.AluOpType.mult, op1=mybir.AluOpType.add,
                    )

                if r == 0:
                    # cold start: A mains first so strip-0's ACT gate (which
                    # lumps all three runs' deps) resolves after 11 mains
                    # instead of 20
                    mains_a(range(0, 11))
                    mains_d(range(0, 9))
                else:
                    mains_a([3, 0, 1, 2])
                    mains_d(range(0, 3))
                    mains_d(range(3, 6))
                    mains_a(range(4, 8))
                    mains_d(range(6, 9))
                    mains_a(range(8, 11))
                act_run(0)
                act_run(1)
                act_run(2)
                dve_run(0)
                dve_run(1)
                dve_run(2)
                # bf16 rowsum of the schraudolph span (4x all-SBUF path)
                nc.vector.tensor_scalar(
                    out=scratch[:], in0=etD[:],
                    scalar1=1.0, scalar2=0.0,
                    op0=mybir.AluOpType.mult, op1=mybir.AluOpType.add,
                    accum_out=rows_all[:, 3 * NSTRIP + r : 3 * NSTRIP + r + 1],
                )
                pending_oh.append((etA, etD))
                if len(pending_oh) > 1:
                    emit_onehots(*pending_oh.pop(0))

            # final batch: D-span first (its dep, the last schraudolph, ends
            # before the last ACT run) so only the A one-hots trail the chain
            eA, eD = pending_oh.pop()
            emit_onehots_d(eD)
            emit_onehots_a(eA)
            nc.vector.tensor_copy(out=colsum_sb[:], in_=ps[0:20, CB : CB + 512])
            nc.sync.dma_start(out=out_rows[:], in_=rows_all[:])
            nc.sync.dma_start(out=out_colsum[:], in_=colsum_sb[:])

    nc.compile()
    return nc


_NC_CACHE = {}
MM_DTYPE = "fp8dr"


def _get_nc(dt_name=MM_DTYPE):
    if dt_name not in _NC_CACHE:
        _NC_CACHE[dt_name] = _build_nc()
    return _NC_CACHE[dt_name]


def _proj_np(z, W1, b1, W2, b2):
    h = z @ W1.T + b1
    h = np.where(h > 0, h, np.expm1(h)).astype(np.float32)
    return (h @ W2.T + b2).astype(np.float32)


def _prepare_operands(z_mp, z_sc, W1, b1, W2, b2):
    zp1 = _proj_np(z_mp.astype(np.float32), W1, b1, W2, b2)
    zp2 = _proj_np(z_sc.astype(np.float32), W1, b1, W2, b2)
    n1 = np.sqrt(np.sum(zp1 * zp1, axis=1, keepdims=True)).astype(np.float32)
    n2 = np.sqrt(np.sum(zp2 * zp2, axis=1, keepdims=True)).astype(np.float32)
    a = (zp1 / n1).astype(np.float32)
    b = (zp2 / (n2 * np.float32(TAU))).astype(np.float32)
    dots = np.sum(a.astype(np.float64) * b, axis=1)  # exact diag logits
    return a, b, dots


def _make_in_maps(a, b):
    np8 = mybir.dt.np(fp8)
    np16 = mybir.dt.np(bf16)
    a_pad = np.zeros((NCORES * RPC, D), np.float32)
    a_pad[:N] = a * SA
    a8 = a_pad.astype(np8)
    b8 = (b * SB).astype(np8)

    # lhsT per core: [4, (strip, t, m)] with element [k, r, t, m] = a8[r*128+m, 4t+k]
    lhs_all = (
        a8.reshape(NCORES, NSTRIP, 128, 2, 4)   # [core, r, m, t, k]
        .transpose(0, 4, 1, 3, 2)               # [core, k, r, t, m]
        .reshape(NCORES, 4, 2 * 128 * NSTRIP)
    )
    # rhsT blocked per 512-tile in RHS_ORDER: [4, (tile, t, n)]
    parts = []
    for t in RHS_ORDER:
        w = TILE_W[t]
        sub = b8[512 * t : 512 * t + w]          # [w, 8]
        parts.append(sub.reshape(w, 2, 4).transpose(2, 1, 0).reshape(4, 2 * w))
    rhsT = np.ascontiguousarray(np.concatenate(parts, axis=1))

    # one-hot E matrices: colsum row = global tile index
    Edr = np.zeros((128, 5, 2, 32), np8)
    for p in range(5):
        Edr[:, p, 0, A_TILES[2 * p]] = 1.0
        Edr[:, p, 1, A_TILES[2 * p + 1]] = 1.0
    ebdr = np.ascontiguousarray(Edr.reshape(128, 320))
    Es8 = np.zeros((128, 20), np8)
    Es8[:, A_TILES[10]] = 1.0
    Ebf = np.zeros((128, 9, 20), np16)
    for j in range(9):
        Ebf[:, j, D_TILES[j]] = 1.0
    ebbf = np.ascontiguousarray(Ebf.reshape(128, 180))

    return [
        {"lhsT": np.ascontiguousarray(lhs_all[k]), "rhsT": rhsT,
         "ebdr": ebdr, "ebs8": Es8, "ebbf": ebbf}
        for k in range(NCORES)
    ]


def _finalize(res, dots):
    # row sums: ACT partials (3 slots/strip) + DVE partial
    rows = []
    for k in range(NCORES):
        m = np.asarray(res[k]["out_rows"]).astype(np.float64)  # [128, 40]
        acts = m[:, : 3 * NSTRIP].reshape(128, NSTRIP, 3).sum(axis=2)  # [128, r]
        dves = m[:, 3 * NSTRIP :]                              # [128, r]
        rows.append((acts + dves).T.reshape(-1))               # row-major r*128+p
    row_sum = np.concatenate(rows)[:N] + EPS

    # col sums: sum cores; colsum row = global tile index
    cs = np.sum([np.asarray(res[k]["out_colsum"]) for k in range(NCORES)], axis=0)
    cs = cs.astype(np.float64)  # [20, 512]
    col_sum = np.empty(N, np.float64)
    for t in range(NT):
        w = TILE_W[t]
        col_sum[512 * t : 512 * t + w] = cs[t, :w]
    # zero pad rows contribute exp(0)=1 in ACT cols and bf16 bits of
    # round(B16) in DVE cols
    v0 = float(np.array([round(B16)], np.int16).view(mybir.dt.np(bf16))[0])
    a_cols = np.zeros(N, bool)
    for t in A_TILES:
        a_cols[512 * t : 512 * t + TILE_W[t]] = True
    pad_col = np.where(a_cols, 1.0, v0)
    col_sum = col_sum - ROW_PAD * pad_col + EPS

    diag = np.exp(dots)
    lori_mp = -np.mean(np.log(diag / row_sum))
    lori_sc = -np.mean(np.log(diag / col_sum))
    return np.float32(LAM * lori_mp + (1.0 - LAM) * lori_sc)


def kernel(z_mp, z_sc, W1, b1, W2, b2):
    a, b, dots = _prepare_operands(z_mp, z_sc, W1, b1, W2, b2)
    in_maps = _make_in_maps(a, b)
    nc = _get_nc()
    res = run_bass_kernel_spmd(nc, in_maps, list(range(NCORES))).results
    return _finalize(res, dots)
